# revision 13
# baseline (speedup 1.0000x reference)
"""2-layer GAT (graph attention) on 8 Trainium2 NeuronCores.

Strategy (dst-partitioned, per the 1D graph partitioning scheme):
 - Node tables: G1 = [f=x@W1 | el | er] for all nodes, G2 likewise for layer 1.
   Each core projects a strip of nodes, then AllGather -> full table on
   every core.
 - Per core, dst nodes are split into 128-row blocks. Edges are bucketed by
   (dst block, src-row range) on the host; per-edge rows of the node table
   are fetched with dma_gather (int16 indices force <=32768-row ranges).
 - Edge softmax: ee = exp(leaky_relu(el[src]+er[dst])); aggregation is a
   one-hot matmul: psum[dst,:] += S.T @ [ee*f | ee], where S[e,d]=1{dloc[e]==d}
   is built on-chip with is_equal against an iota row. er[dst] per edge is
   produced without a gather: er_edge = St.T @ er_block where St is the
   transposed one-hot (built from a partition-broadcast copy of dloc).
 - Normalization (divide by sum ee), bias, relu in the block epilogue; the
   layer-2 projection consumes the PE-transposed h tile immediately, so h
   never round-trips through DRAM.
 - dst assignment is chosen so each core's layer-1 dst rows are a prefix of
   its own layer-0 dst rows (er2 comes from the core's own layer-0 epilogue,
   keeping the program SPMD-uniform).
"""

import math
import numpy as np

P = 128

CFG = dict(
    NC=8,
    N0=100000, N1=50000, N2=25000,
    H1=4, D1=64, H2=1, D2=64,
    F0=256, SLOPE=0.2,
    RANGE=32768,
)


def _ceil_to(x, m):
    return -(-x // m) * m


def _derive(cfg):
    NC = cfg["NC"]
    d = {}
    d["n0pc"] = _ceil_to(cfg["N0"], NC * P) // NC
    d["N0P"] = NC * d["n0pc"]
    d["n2pc"] = _ceil_to(cfg["N2"], NC * P) // NC
    d["N2P"] = NC * d["n2pc"]
    rem = cfg["N1"] - d["N2P"]
    assert rem > 0, "layout assumes N1 > padded N2"
    d["bpc"] = _ceil_to(rem, NC * P) // NC
    d["n1pc"] = d["n2pc"] + d["bpc"]
    d["N1P"] = NC * d["n1pc"]
    d["nb0"] = d["n1pc"] // P      # layer-0 dst blocks per core
    d["nb1"] = d["n2pc"] // P      # layer-1 dst blocks per core
    def _split(n, k):
        k = max(1, min(k, n))
        q, r = divmod(n, k)
        return [q + (i < r) for i in range(k)]
    # AllGather slices double as int16 index ranges: slice k of the strip is
    # gathered into its own table of NC*slice_rows (< 32768) rows.
    d["sl0"] = _split(d["n0pc"] // P, -(-d["N0P"] // cfg["RANGE"]))
    d["sl1"] = _split(d["nb0"], -(-d["N1P"] // cfg["RANGE"]))
    for ch in d["sl0"]:
        assert cfg["NC"] * ch * P < 32768
    for ch in d["sl1"]:
        assert cfg["NC"] * ch * P < 32768
    d["nr0"] = len(d["sl0"])
    d["nr1"] = len(d["sl1"])
    d["TW1"] = cfg["F0"] + 64      # 256 f + 4 el + 4 er + pad -> 320 (1280B)
    d["PJ1"] = cfg["F0"] + 2 * cfg["H1"]   # 264 projected cols
    d["TW2"] = 128                 # 64 f2 + el2 + er2 + pad -> 128 (512B)
    d["PJ2"] = cfg["D2"] + 2   # 66: [f2 | el2 | er2]
    d["mk1"] = cfg["F0"] // P      # K chunks for layer-0 projection (2)
    d["mk2"] = (cfg["H1"] * cfg["D1"]) // P  # K chunks for layer-1 proj (2)
    return d


def _l0_owner_local(dst, d):
    """layer-0 dst node -> (core, local row). A-part = first n2pc rows of each
    core (aligned with the layer-1 dst range), B-part = the rest."""
    n2pc, bpc, N2P = d["n2pc"], d["bpc"], d["N2P"]
    a = dst < N2P
    c = np.where(a, dst // n2pc, (dst - N2P) // bpc)
    loc = np.where(a, dst % n2pc, n2pc + (dst - N2P) % bpc)
    return c.astype(np.int64), loc.astype(np.int64)


def _slice_map(loc_chunks, slices):
    """per-chunk slice id and base chunk within slice"""
    sid = np.zeros(sum(slices), np.int64)
    base = np.zeros(sum(slices), np.int64)
    k = 0
    for i, n in enumerate(slices):
        sid[k:k + n] = i
        base[k:k + n] = k
        k += n
    return sid, base


def _prep_edges(rng, sl, dst_c, dst_loc, nb, nr, NC):
    """Bucket edges by (core, block, range); pad each bucket to a multiple of
    128 slots, chunk counts maxed across cores (SPMD needs one program).

    Returns (segments, totals, per_core) where
      segments: list of (b, r, local_chunk0, nch, idxcol0) shared by all cores
      totals:   (total_chunks, chunk_base[nb])
      per_core: list of dicts with idx16 [128, S/16], dloc_col [128, C],
                dloc_bc [1, S]
    """
    blk = dst_loc // P
    dl = (dst_loc % P).astype(np.float32)
    assert sl.max(initial=0) < 32768

    counts = np.zeros((NC, nb, nr), np.int64)
    np.add.at(counts, (dst_c, blk, rng), 1)
    maxch = -(-counts.max(axis=0) // P)          # [nb, nr]
    Cb = maxch.sum(axis=1)                       # [nb]
    chunk_base = np.concatenate([[0], np.cumsum(Cb)])
    total_chunks = int(Cb.sum())
    total_slots = total_chunks * P

    # shared segment table + idx16 column offsets
    segments = []
    idxcol = 0
    seg_info = {}
    for b in range(nb):
        lc = 0
        for r in range(nr):
            nch = int(maxch[b, r])
            if nch == 0:
                continue
            seg_info[(b, r)] = (lc, nch, idxcol)
            # split into gather calls of at most 8 chunks (1024 indices)
            off = 0
            while off < nch:
                n = min(8, nch - off)
                segments.append((b, r, lc + off, n, idxcol + off * 8))
                off += n
            lc += nch
            idxcol += nch * 8          # nch*128/16 int16 columns
        assert lc == Cb[b]

    order = np.lexsort((rng, blk, dst_c))
    key = (dst_c * nb + blk) * nr + rng
    skey = key[order]
    bounds = np.searchsorted(skey, np.arange(NC * nb * nr + 1))

    per_core = []
    for c in range(NC):
        idx16 = np.zeros((16, total_slots // 16), np.int16)
        dloc_col = np.full((P, total_chunks), 999.0, np.float32)
        dloc_bc = np.full(total_slots, 999.0, np.float32)
        for b in range(nb):
            for r in range(nr):
                if (b, r) not in seg_info:
                    continue
                lc, nch, col0 = seg_info[(b, r)]
                k = (c * nb + b) * nr + r
                e = order[bounds[k]:bounds[k + 1]]
                n = len(e)
                nslot = nch * P
                assert n <= nslot
                sidx = np.zeros(nslot, np.int64)
                sidx[:n] = sl[e]
                sdl = np.full(nslot, 999.0, np.float32)
                sdl[:n] = dl[e]
                idx16[:, col0:col0 + nch * 8] = (
                    sidx.reshape(-1, 16).T.astype(np.int16))
                g0 = chunk_base[b] + lc
                dloc_col[:, g0:g0 + nch] = sdl.reshape(nch, P).T
                dloc_bc[g0 * P:(g0 + nch) * P] = sdl
        per_core.append(dict(
            idx16=np.tile(idx16, (8, 1)),
            dloc_col=dloc_col,
            dloc_bc=dloc_bc[None, :],
        ))
    return segments, (total_chunks, chunk_base), per_core


def _host_prep(inputs, cfg):
    d = _derive(cfg)
    NC = cfg["NC"]
    F0, H1, D1, D2 = cfg["F0"], cfg["H1"], cfg["D1"], cfg["D2"]

    x = np.asarray(inputs["x"], np.float32)
    xp = np.zeros((d["N0P"], F0), np.float32)
    xp[:cfg["N0"]] = x

    # weight packing: G1 cols = [f | el | er], same for layer 2
    al1 = np.asarray(inputs["al1"], np.float32)
    ar1 = np.asarray(inputs["ar1"], np.float32)
    A_l = np.zeros((H1 * D1, H1), np.float32)
    A_r = np.zeros((H1 * D1, H1), np.float32)
    for h in range(H1):
        A_l[h * D1:(h + 1) * D1, h] = al1[h]
        A_r[h * D1:(h + 1) * D1, h] = ar1[h]
    W1 = np.asarray(inputs["W1"], np.float32)
    W1e = np.concatenate([W1, W1 @ A_l, W1 @ A_r], axis=1)  # [F0, PJ1]

    W2 = np.asarray(inputs["W2"], np.float32)
    al2 = np.asarray(inputs["al2"], np.float32).reshape(-1, 1)
    ar2 = np.asarray(inputs["ar2"], np.float32).reshape(-1, 1)
    W2e = np.concatenate([W2, W2 @ al2, W2 @ ar2], axis=1)  # [256, 66]

    # edges, layer 0: src node n lives in AG-slice k of its owner's strip;
    # gather table k has rows [owner * slice_rows + offset]
    sid0, sbase0 = _slice_map(d["n0pc"] // P, d["sl0"])
    e0s = np.asarray(inputs["e0_src"], np.int64)
    e0d = np.asarray(inputs["e0_dst"], np.int64)
    oc = e0s // d["n0pc"]
    orow = e0s % d["n0pc"]
    och = orow // P
    rng0 = sid0[och]
    slr0 = np.array(d["sl0"])[rng0] * P
    sl0 = oc * slr0 + (orow - sbase0[och] * P)
    c0, loc0 = _l0_owner_local(e0d, d)
    seg0, tot0, pc0 = _prep_edges(rng0, sl0, c0, loc0, d["nb0"], d["nr0"], NC)

    # edges, layer 1: src node n sits at local row local0(n) of owner0(n)'s
    # G2 strip, in AG-slice k of the layer-1 split
    sid1, sbase1 = _slice_map(d["nb0"], d["sl1"])
    e1s = np.asarray(inputs["e1_src"], np.int64)
    e1d = np.asarray(inputs["e1_dst"], np.int64)
    sc, sloc = _l0_owner_local(e1s, d)
    sch = sloc // P
    rng1 = sid1[sch]
    slr1 = np.array(d["sl1"])[rng1] * P
    sl1 = sc * slr1 + (sloc - sbase1[sch] * P)
    c1 = e1d // d["n2pc"]
    loc1 = e1d % d["n2pc"]
    seg1, tot1, pc1 = _prep_edges(rng1, sl1, c1, loc1, d["nb1"], d["nr1"], NC)

    b1 = np.asarray(inputs["b1"], np.float32)
    b2 = np.asarray(inputs["b2"], np.float32)

    consts = dict(
        W1e_a=W1e[:P].copy(), W1e_b=W1e[P:].copy(),
        W2e_a=W2e[:P].copy(), W2e_b=W2e[P:].copy(),
        IOTA_ROW=np.tile(np.arange(P, dtype=np.float32), (P, 1)).copy(),
        IOTA_COL=np.arange(P, dtype=np.float32)[:, None].copy(),
        IDENT=np.eye(P, dtype=np.float32),
        B1T=np.tile(b1[None, :], (P, 1)).copy(),
        B2T=np.tile(b2[None, :], (P, 1)).copy(),
    )

    in_maps = []
    for c in range(NC):
        xT = np.ascontiguousarray(xp[c * d["n0pc"]:(c + 1) * d["n0pc"]].T)
        rows_a = np.arange(c * d["n2pc"], (c + 1) * d["n2pc"])
        rows_b = d["N2P"] + np.arange(c * d["bpc"], (c + 1) * d["bpc"])
        xTd = np.ascontiguousarray(xp[np.concatenate([rows_a, rows_b])].T)
        m = dict(consts)
        m["xT"] = xT
        m["xTd"] = xTd
        m["IDX0"] = pc0[c]["idx16"]
        m["DLC0"] = pc0[c]["dloc_col"]
        m["DLB0"] = pc0[c]["dloc_bc"]
        m["IDX1"] = pc1[c]["idx16"]
        m["DLC1"] = pc1[c]["dloc_col"]
        m["DLB1"] = pc1[c]["dloc_bc"]
        in_maps.append(m)

    meta = dict(d=d, seg0=seg0, tot0=tot0, seg1=seg1, tot1=tot1)
    return in_maps, meta


def _build(cfg, meta, stages="all"):
    import concourse.bass as bass
    import concourse.bacc as bacc
    import concourse.mybir as mybir
    import concourse.tile as tile

    d = meta["d"]
    NC = cfg["NC"]
    F0, H1, D1, D2 = cfg["F0"], cfg["H1"], cfg["D1"], cfg["D2"]
    SLOPE = cfg["SLOPE"]
    TW1, PJ1, TW2, PJ2 = d["TW1"], d["PJ1"], d["TW2"], d["PJ2"]
    nb0, nb1 = d["nb0"], d["nb1"]
    seg0, (C0, cb0) = meta["seg0"], meta["tot0"]
    seg1, (C1, cb1) = meta["seg1"], meta["tot1"]
    S0, S1 = C0 * P, C1 * P
    fdt = mybir.dt.float32
    AL = mybir.AluOpType

    nc = bacc.Bacc("TRN2", target_bir_lowering=False, debug=False,
                   num_devices=NC)

    def din(name, shape, dt=fdt):
        return nc.dram_tensor(name, shape, dt, kind="ExternalInput")

    xT = din("xT", [F0, d["n0pc"]])
    xTd = din("xTd", [F0, d["n1pc"]])
    W1e_a = din("W1e_a", [P, PJ1]); W1e_b = din("W1e_b", [P, PJ1])
    W2e_a = din("W2e_a", [P, PJ2]); W2e_b = din("W2e_b", [P, PJ2])
    IOTA_ROW = din("IOTA_ROW", [P, P]); IOTA_COL = din("IOTA_COL", [P, 1])
    IDENT = din("IDENT", [P, P])
    B1T = din("B1T", [P, F0]); B2T = din("B2T", [P, D2])
    IDX0 = din("IDX0", [P, S0 // 16], mybir.dt.int16)
    DLC0 = din("DLC0", [P, C0]); DLB0 = din("DLB0", [1, S0])
    IDX1 = din("IDX1", [P, S1 // 16], mybir.dt.int16)
    DLC1 = din("DLC1", [P, C1]); DLB1 = din("DLB1", [1, S1])
    OUT = nc.dram_tensor("OUT", [d["n2pc"], D2], fdt, kind="ExternalOutput")

    G1S = nc.dram_tensor("G1S", [d["n0pc"], TW1], fdt)
    G2S = nc.dram_tensor("G2S", [d["n1pc"], TW2], fdt)
    G1F = []
    base = 0
    for k, ch in enumerate(d["sl0"]):
        G1F.append(nc.dram_tensor(f"G1F{k}", [NC * ch * P, TW1], fdt,
                                  addr_space="Shared"))
        base += ch
    G2F = []
    for k, ch in enumerate(d["sl1"]):
        G2F.append(nc.dram_tensor(f"G2F{k}", [NC * ch * P, TW2], fdt,
                                  addr_space="Shared"))
    sl0_base = np.concatenate([[0], np.cumsum(d["sl0"])]).astype(int)
    sl1_base = np.concatenate([[0], np.cumsum(d["sl1"])]).astype(int)

    def bcast_row(dram, s0, n):
        """DRAM [1, N] slice -> AP broadcast across 128 partitions."""
        ap = dram[0:1, s0:s0 + n]
        return bass.AP(ap.tensor, ap.offset, [[0, P], [1, n]])

    seg_by_block0 = {}
    for (b, r, lc, nch, col0) in seg0:
        seg_by_block0.setdefault(b, []).append((r, lc, nch, col0))
    seg_by_block1 = {}
    for (b, r, lc, nch, col0) in seg1:
        seg_by_block1.setdefault(b, []).append((r, lc, nch, col0))

    with tile.TileContext(nc) as tc:
        with tc.tile_pool(name="const", bufs=1) as cp, \
             tc.tile_pool(name="work", bufs=2) as wp, \
             tc.tile_pool(name="chk", bufs=4) as kp, \
             tc.tile_pool(name="psum", bufs=2, space="PSUM") as pp, \
             tc.tile_pool(name="psE", bufs=2, space="PSUM") as pe, \
             tc.tile_pool(name="psT", bufs=3, space="PSUM") as pt:

            def const_tile(name, dram, shape, dt=fdt):
                t = cp.tile(shape, dt, tag=name)
                nc.sync.dma_start(out=t[:], in_=dram[:, :])
                return t

            w1a = const_tile("w1a", W1e_a, [P, PJ1])
            w1b = const_tile("w1b", W1e_b, [P, PJ1])
            w2a = const_tile("w2a", W2e_a, [P, PJ2])
            w2b = const_tile("w2b", W2e_b, [P, PJ2])
            iorow = const_tile("iorow", IOTA_ROW, [P, P])
            iocol = const_tile("iocol", IOTA_COL, [P, 1])
            ident = const_tile("ident", IDENT, [P, P])
            b1t = const_tile("b1t", B1T, [P, F0])
            b2t = const_tile("b2t", B2T, [P, D2])
            idx0 = const_tile("idx0", IDX0, [P, S0 // 16], mybir.dt.int16)
            dlc0 = const_tile("dlc0", DLC0, [P, C0])
            idx1 = const_tile("idx1", IDX1, [P, S1 // 16], mybir.dt.int16)
            dlc1 = const_tile("dlc1", DLC1, [P, C1])
            er_sb = cp.tile([P, nb0, H1], fdt, tag="er_sb")
            er2_sb = cp.tile([P, nb1, 1], fdt, tag="er2_sb")

            # ---- layer-0 projection: G1 strip = [f | el | er ],
            # AllGather each slice as soon as it is projected ----
            for sli, ch in enumerate(d["sl0"]):
                for m in range(int(sl0_base[sli]), int(sl0_base[sli]) + ch):
                    ps = pp.tile([P, PJ1], fdt, tag="agg", space="PSUM")
                    for k in range(d["mk1"]):
                        xt = kp.tile([P, P], fdt, tag="xt")
                        nc.sync.dma_start(
                            out=xt[:],
                            in_=xT[k * P:(k + 1) * P, m * P:(m + 1) * P])
                        nc.tensor.matmul(out=ps[:], lhsT=xt[:],
                                         rhs=(w1a if k == 0 else w1b)[:],
                                         start=(k == 0),
                                         stop=(k == d["mk1"] - 1))
                    sb = kp.tile([P, TW1], fdt, tag="pjsb")
                    nc.scalar.copy(out=sb[:, 0:PJ1], in_=ps[:])
                    nc.vector.memset(sb[:, PJ1:TW1], 0.0)
                    nc.sync.dma_start(out=G1S[m * P:(m + 1) * P, :], in_=sb[:])
                r0 = int(sl0_base[sli]) * P
                r1 = r0 + ch * P
                nc.gpsimd.collective_compute(
                    "AllGather", AL.bypass,
                    replica_groups=[list(range(NC))],
                    ins=[G1S[r0:r1, :]], outs=[G1F[sli][:, :]])

            # ---- own-dst er projection (kept in SBUF) ----
            for b in range(nb0):
                ps = pe.tile([P, H1], fdt, tag="er", space="PSUM")
                for k in range(d["mk1"]):
                    xt = kp.tile([P, P], fdt, tag="xt")
                    nc.sync.dma_start(
                        out=xt[:],
                        in_=xTd[k * P:(k + 1) * P, b * P:(b + 1) * P])
                    w = (w1a if k == 0 else w1b)
                    nc.tensor.matmul(out=ps[:], lhsT=xt[:],
                                     rhs=w[:, F0 + H1:F0 + 2 * H1],
                                     start=(k == 0), stop=(k == d["mk1"] - 1))
                nc.scalar.copy(out=er_sb[:, b, :], in_=ps[:])

            # ---- layer-0 blocks ----
            for b in range(nb0 if stages != "proj" else 0):
                segs = seg_by_block0.get(b, [])
                Cb = sum(nch for (_, _, nch, _) in segs)
                if Cb == 0:
                    # still must produce zero h -> g2 row
                    Cb = 0
                R = wp.tile([P, max(Cb, 1), TW1], fdt, tag="R")
                for (r, lc, nch, col0) in segs:
                    nsl = nch * P
                    nc.gpsimd.dma_gather(
                        R[:, lc:lc + nch, :], G1F[r][:, :],
                        idx0[:, col0:col0 + nch * 8], nsl, nsl, TW1)
                dlb = wp.tile([P, max(Cb, 1) * P], fdt, tag="dlb")
                if Cb:
                    nc.sync.dma_start(
                        out=dlb[:, :Cb * P],
                        in_=bcast_row(DLB0, int(cb0[b]) * P, Cb * P))
                ps = pp.tile([P, PJ1], fdt, tag="agg", space="PSUM")
                if Cb:
                    gc0 = int(cb0[b])
                    sall = wp.tile([P, Cb, P], fdt, tag="sall")
                    nc.vector.tensor_tensor(
                        out=sall[:],
                        in0=dlc0[:, gc0:gc0 + Cb].unsqueeze(2)
                            .to_broadcast([P, Cb, P]),
                        in1=iorow[:].unsqueeze(1).to_broadcast([P, Cb, P]),
                        op=AL.is_equal)
                    stall = wp.tile([P, Cb, P], fdt, tag="stall")
                    nc.vector.tensor_tensor(
                        out=stall[:],
                        in0=iocol[:].unsqueeze(2).to_broadcast([P, Cb, P]),
                        in1=dlb[:, :Cb * P].rearrange("p (c e) -> p c e", e=P),
                        op=AL.is_equal)
                    erall = pe.tile([P, Cb * H1], fdt, tag="er", space="PSUM")
                    for ci in range(Cb):
                        nc.tensor.matmul(out=erall[:, ci * H1:(ci + 1) * H1],
                                         lhsT=stall[:, ci, :],
                                         rhs=er_sb[:, b, :],
                                         start=True, stop=True)
                    eall = kp.tile([P, Cb, H1], fdt, tag="eall")
                    nc.vector.tensor_tensor(
                        out=eall[:], in0=R[:, :Cb, F0:F0 + H1],
                        in1=erall[:].rearrange("p (c h) -> p c h", h=H1),
                        op=AL.add)
                    e2all = kp.tile([P, Cb, H1], fdt, tag="e2all")
                    nc.vector.tensor_scalar(out=e2all[:], in0=eall[:],
                                            scalar1=SLOPE, scalar2=None,
                                            op0=AL.mult)
                    nc.vector.tensor_tensor(out=e2all[:], in0=eall[:],
                                            in1=e2all[:], op=AL.max)
                    msg = wp.tile([P, Cb, F0 + H1], fdt, tag="msg")
                    nc.scalar.activation(out=msg[:, :, F0:F0 + H1],
                                         in_=e2all[:],
                                         func=mybir.ActivationFunctionType.Exp)
                    nc.vector.tensor_tensor(
                        out=msg[:, :, 0:F0].rearrange(
                            "p c (h j) -> p c h j", h=H1),
                        in0=R[:, :Cb, 0:F0].rearrange(
                            "p c (h j) -> p c h j", h=H1),
                        in1=msg[:, :, F0:F0 + H1].unsqueeze(3)
                            .to_broadcast([P, Cb, H1, D1]),
                        op=AL.mult)
                    for ci in range(Cb):
                        nc.tensor.matmul(out=ps[:, 0:F0 + H1],
                                         lhsT=sall[:, ci, :],
                                         rhs=msg[:, ci, :], start=(ci == 0),
                                         stop=(ci == Cb - 1))
                if Cb == 0:
                    z = kp.tile([P, F0 + H1], fdt, tag="msg")
                    nc.vector.memset(z[:], 0.0)
                    s = kp.tile([P, P], fdt, tag="s")
                    nc.vector.memset(s[:], 0.0)
                    nc.tensor.matmul(out=ps[:, 0:F0 + H1], lhsT=s[:],
                                     rhs=z[:], start=True, stop=True)
                # epilogue: normalize, bias, relu
                rr = kp.tile([P, H1], fdt, tag="rr")
                nc.vector.tensor_scalar(out=rr[:], in0=ps[:, F0:F0 + H1],
                                        scalar1=1e-30, scalar2=None, op0=AL.add)
                nc.vector.reciprocal(out=rr[:], in_=rr[:])
                hsb = wp.tile([P, F0], fdt, tag="hsb")
                nc.vector.tensor_tensor(
                    out=hsb[:].rearrange("p (h j) -> p h j", h=H1),
                    in0=ps[:, 0:F0].rearrange("p (h j) -> p h j", h=H1),
                    in1=rr[:].unsqueeze(2).to_broadcast([P, H1, D1]),
                    op=AL.mult)
                nc.vector.tensor_tensor(out=hsb[:], in0=hsb[:], in1=b1t[:],
                                        op=AL.add)
                nc.scalar.activation(out=hsb[:], in_=hsb[:],
                                     func=mybir.ActivationFunctionType.Relu)
                g2p = pt.tile([P, PJ2], fdt, tag="epi", space="PSUM")
                for k in range(d["mk2"]):
                    tp = pt.tile([P, P], fdt, tag="epi", space="PSUM")
                    nc.tensor.transpose(out=tp[:],
                                        in_=hsb[:, k * P:(k + 1) * P],
                                        identity=ident[:])
                    hT = kp.tile([P, P], fdt, tag="hT")
                    nc.scalar.copy(out=hT[:], in_=tp[:])
                    nc.tensor.matmul(out=g2p[:], lhsT=hT[:],
                                     rhs=(w2a if k == 0 else w2b)[:],
                                     start=(k == 0), stop=(k == d["mk2"] - 1))
                g2sb = kp.tile([P, TW2], fdt, tag="g2sb")
                nc.scalar.copy(out=g2sb[:, 0:PJ2], in_=g2p[:])
                nc.vector.memset(g2sb[:, PJ2:TW2], 0.0)
                nc.sync.dma_start(out=G2S[b * P:(b + 1) * P, :], in_=g2sb[:])
                if b < nb1:
                    nc.scalar.copy(out=er2_sb[:, b, :],
                                   in_=g2sb[:, PJ2 - 1:PJ2])
                for sli in range(d["nr1"]):
                    if b == int(sl1_base[sli + 1]) - 1 and stages != "l0":
                        r0 = int(sl1_base[sli]) * P
                        r1 = int(sl1_base[sli + 1]) * P
                        nc.gpsimd.collective_compute(
                            "AllGather", AL.bypass,
                            replica_groups=[list(range(NC))],
                            ins=[G2S[r0:r1, :]], outs=[G2F[sli][:, :]])

            # ---- layer-1 blocks ----
            for b in range(nb1 if stages in ("all", "nol1g") else 0):
                segs = seg_by_block1.get(b, [])
                Cb = sum(nch for (_, _, nch, _) in segs)
                R = wp.tile([P, max(Cb, 1), TW2], fdt, tag="R2")
                if stages == "nol1g":
                    nc.vector.memset(R[:], 0.0)
                else:
                    for (r, lc, nch, col0) in segs:
                        nsl = nch * P
                        nc.gpsimd.dma_gather(
                            R[:, lc:lc + nch, :], G2F[r][:, :],
                            idx1[:, col0:col0 + nch * 8], nsl, nsl, TW2)
                dlb = wp.tile([P, max(Cb, 1) * P], fdt, tag="dlb2")
                if Cb:
                    nc.sync.dma_start(
                        out=dlb[:, :Cb * P],
                        in_=bcast_row(DLB1, int(cb1[b]) * P, Cb * P))
                ps = pp.tile([P, D2 + 1], fdt, tag="agg", space="PSUM")
                if Cb:
                    gc0 = int(cb1[b])
                    sall = wp.tile([P, Cb, P], fdt, tag="sall")
                    nc.vector.tensor_tensor(
                        out=sall[:],
                        in0=dlc1[:, gc0:gc0 + Cb].unsqueeze(2)
                            .to_broadcast([P, Cb, P]),
                        in1=iorow[:].unsqueeze(1).to_broadcast([P, Cb, P]),
                        op=AL.is_equal)
                    stall = wp.tile([P, Cb, P], fdt, tag="stall")
                    nc.vector.tensor_tensor(
                        out=stall[:],
                        in0=iocol[:].unsqueeze(2).to_broadcast([P, Cb, P]),
                        in1=dlb[:, :Cb * P].rearrange("p (c e) -> p c e", e=P),
                        op=AL.is_equal)
                    erall = pe.tile([P, Cb], fdt, tag="er", space="PSUM")
                    for ci in range(Cb):
                        nc.tensor.matmul(out=erall[:, ci:ci + 1],
                                         lhsT=stall[:, ci, :],
                                         rhs=er2_sb[:, b, :],
                                         start=True, stop=True)
                    eall = kp.tile([P, Cb, 1], fdt, tag="eall")
                    nc.vector.tensor_tensor(
                        out=eall[:], in0=R[:, :Cb, D2:D2 + 1],
                        in1=erall[:].unsqueeze(2), op=AL.add)
                    e2all = kp.tile([P, Cb, 1], fdt, tag="e2all")
                    nc.vector.tensor_scalar(out=e2all[:], in0=eall[:],
                                            scalar1=SLOPE, scalar2=None,
                                            op0=AL.mult)
                    nc.vector.tensor_tensor(out=e2all[:], in0=eall[:],
                                            in1=e2all[:], op=AL.max)
                    msg = wp.tile([P, Cb, D2 + 1], fdt, tag="msg2")
                    nc.scalar.activation(out=msg[:, :, D2:D2 + 1],
                                         in_=e2all[:],
                                         func=mybir.ActivationFunctionType.Exp)
                    nc.vector.tensor_tensor(
                        out=msg[:, :, 0:D2],
                        in0=R[:, :Cb, 0:D2],
                        in1=msg[:, :, D2:D2 + 1].to_broadcast([P, Cb, D2]),
                        op=AL.mult)
                    for ci in range(Cb):
                        nc.tensor.matmul(out=ps[:], lhsT=sall[:, ci, :],
                                         rhs=msg[:, ci, :], start=(ci == 0),
                                         stop=(ci == Cb - 1))
                if Cb == 0:
                    z = kp.tile([P, D2 + 1], fdt, tag="msg2")
                    nc.vector.memset(z[:], 0.0)
                    s = kp.tile([P, P], fdt, tag="s")
                    nc.vector.memset(s[:], 0.0)
                    nc.tensor.matmul(out=ps[:], lhsT=s[:], rhs=z[:],
                                     start=True, stop=True)
                rr = kp.tile([P, 1], fdt, tag="rr")
                nc.vector.tensor_scalar(out=rr[:], in0=ps[:, D2:D2 + 1],
                                        scalar1=1e-30, scalar2=None, op0=AL.add)
                nc.vector.reciprocal(out=rr[:], in_=rr[:])
                osb = kp.tile([P, D2], fdt, tag="osb")
                nc.vector.tensor_scalar(out=osb[:], in0=ps[:, 0:D2],
                                        scalar1=rr[:, 0:1], scalar2=None,
                                        op0=AL.mult)
                nc.vector.tensor_tensor(out=osb[:], in0=osb[:], in1=b2t[:],
                                        op=AL.add)
                nc.sync.dma_start(out=OUT[b * P:(b + 1) * P, :], in_=osb[:])

    nc.compile()
    return nc


def kernel(**inputs):
    from concourse import bass_utils
    cfg = CFG
    in_maps, meta = _host_prep(inputs, cfg)
    nc = _build(cfg, meta)
    res = bass_utils.run_bass_kernel_spmd(
        nc, in_maps, core_ids=list(range(cfg["NC"])))
    d = meta["d"]
    out = np.concatenate([res.results[c]["OUT"] for c in range(cfg["NC"])],
                         axis=0)
    return np.ascontiguousarray(out[:cfg["N2"]]).astype(np.float32)
